# revision 12
# baseline (speedup 1.0000x reference)
"""Multi-head self-attention Trainium2 kernel (Bass/Tile), batch-parallel
over 8 NeuronCores.

Problem (hardcoded): B=8, L=1024, D=1024, H=16, hd=64, f32.
  qkv = x @ w_qkv + b_qkv ; per-head scores = q k^T / 8 ; mask ; softmax ;
  out = (P v) heads-merged @ w_out + b_out.

Sharding: one batch element per core (data parallel); full weights on every
core. No collectives.

Per-core dataflow (all real matmuls in float32r, N=512, f32 PSUM accum):
  - host provides xT (D x L, dim-major) and w_qkv pre-blocked so every DMA is
    contiguous.
  - qkvT[3D x L] = w_qkv^T @ x^T via 24 M-tiles x 8 K-chunks (PSUM accum),
    evacuated with per-partition b_qkv add (DVE) -> SBUF f32r.
  - per head: ST = k^T q (scores TRANSPOSED: [Lk x Lq]) -> exp via ScalarE
    with scale=1/8 and per-partition mask bias (exact masking for free),
    no max-subtraction (inputs bounded; softmax is shift-invariant).
  - V^T per head via PE transposes into one PSUM bank, augmented with a ones
    column so the attention matmul also produces the softmax denominator.
  - O_un^T[65 x Lq] = V'^T E accumulated over Lk chunks. Rows 0:64 are the
    unnormalized head output (dim-major), row 64 the denominator.
  - normalization deferred: OT chunk tiles [128 x L] collect 2 heads' raw
    rows; R = 1/denom broadcast across partitions via SBUF->SBUF DMA;
    one DVE multiply normalizes and rounds to f32r.
  - final = OT^T @ w_out per Lq-tile (+ b_out broadcast add) -> token-major
    output, DMA'd straight to DRAM.
"""

import sys

import numpy as np

try:
    import concourse.bass as bass  # noqa: F401
except Exception:  # pragma: no cover - defensive path setup
    for p in ("/opt/trn_rl_repo", "/opt/pypackages"):
        if p not in sys.path:
            sys.path.insert(0, p)
    import concourse.bass as bass  # noqa: F401

from contextlib import ExitStack

import concourse.tile as tile
from concourse import bacc, mybir
from concourse.bass_utils import run_bass_kernel_spmd
from concourse.masks import make_identity

F32 = mybir.dt.float32
F32R = mybir.dt.float32r

B, L, D = 8, 1024, 1024
H, HD = 16, 64
D3 = 3 * D
N_CORES = 8
PART = 128
NK = D // PART  # 8 contraction chunks
NM = D3 // PART  # 24 qkv output tiles
NLQ = L // PART  # 8 query tiles
NLK = L // PART  # 8 key tiles
MG = 3  # qkv M-tiles per PSUM group


def build_nc(debug=False):
    nc = bacc.Bacc("TRN2", target_bir_lowering=False, debug=False)

    xT = nc.dram_tensor("xT", (D, L), F32, kind="ExternalInput").ap()
    # w_qkv blocked on host: wqkv_blk[m, p, k, c] = w_qkv[k*128 + p, m*128 + c]
    wqkv_blk = nc.dram_tensor(
        "wqkv_blk", (NM, PART, NK, PART), F32, kind="ExternalInput"
    ).ap()
    bqkv = nc.dram_tensor("bqkv", (D3,), F32, kind="ExternalInput").ap()
    wout = nc.dram_tensor("wout", (D, D), F32, kind="ExternalInput").ap()
    bout = nc.dram_tensor("bout", (PART, D), F32, kind="ExternalInput").ap()
    maskb = nc.dram_tensor("maskb", (L,), F32, kind="ExternalInput").ap()
    sel = nc.dram_tensor("sel", (2, PART), F32, kind="ExternalInput").ap()
    Y = nc.dram_tensor("Y", (L, D), F32, kind="ExternalOutput").ap()
    dbg = {}
    if debug:
        for nm, shp in [
            ("dbg_q", (PART, L)), ("dbg_k", (PART, L)), ("dbg_v", (PART, L)),
            ("dbg_e", (PART, L)), ("dbg_vt", (PART, NLK * (HD + 1))),
            ("dbg_po", (PART, L)), ("dbg_rt", (PART, L)), ("dbg_ot", (PART, L)),
        ]:
            dbg[nm] = nc.dram_tensor(nm, shp, F32, kind="ExternalOutput").ap()

    with tile.TileContext(nc) as tc, ExitStack() as ctx:
        singles = ctx.enter_context(tc.tile_pool(name="singles", bufs=1))

        # two stacked 64x64 identities so V-transposes work from either
        # base partition (matmul requires lhsT/rhs at the same base)
        ident = singles.tile([PART, HD], F32)
        make_identity(nc, ident[0:HD, 0:HD])
        make_identity(nc, ident[HD:PART, 0:HD])
        ones_sb = singles.tile([PART, 1], F32)
        nc.vector.memset(ones_sb[:], 1.0)
        sel_sb = singles.tile([2, PART], F32R)
        nc.sync.dma_start(sel_sb[:], sel[:, :].bitcast(F32R))
        bqkv_sb = singles.tile([PART, NM], F32)
        nc.sync.dma_start(bqkv_sb[:], bqkv.rearrange("(c p) -> p c", p=PART))
        mb_sb = singles.tile([PART, NLK], F32)
        nc.sync.dma_start(mb_sb[:], maskb.rearrange("(c p) -> p c", p=PART))
        bout_sb = singles.tile([PART, D], F32)
        nc.sync.dma_start(bout_sb[:], bout[:, :])

        # ---- tiles that must survive across phases ----
        qkvT_pool = ctx.enter_context(tc.tile_pool(name="qkvT", bufs=1))
        qkvT = []
        for m in range(NM):
            t = qkvT_pool.tile([PART, L], F32R, tag=f"qkvT{m}")
            qkvT.append(t)

        ot_pool = ctx.enter_context(tc.tile_pool(name="otpool", bufs=1))
        ot_fin = []
        for j in range(NK):
            t = ot_pool.tile([PART, L], F32R, tag=f"ot{j}")
            ot_fin.append(t)

        # ================= phase 1: qkv projection =================
        with (
            tc.tile_pool(name="xt", bufs=1) as xt_pool,
            tc.tile_pool(name="wblk", bufs=2 * MG) as wblk_pool,
            tc.tile_pool(name="pq", bufs=MG, space="PSUM") as pq_pool,
        ):
            xt = []
            for k in range(NK):
                t = xt_pool.tile([PART, L], F32R, tag=f"xt{k}")
                nc.sync.dma_start(t[:], xT[k * PART : (k + 1) * PART, :].bitcast(F32R))
                xt.append(t)

            for g in range(NM // MG):
                ms = [g * MG + i for i in range(MG)]
                wtiles = {}
                for m in ms:
                    wt = wblk_pool.tile([PART, NK * PART], F32R, tag="wblk")
                    nc.sync.dma_start(
                        wt[:],
                        wqkv_blk[m].rearrange("p k c -> p (k c)").bitcast(F32R),
                    )
                    wtiles[m] = wt
                pts = {}
                for m in ms:
                    pt = pq_pool.tile([PART, L], F32, tag="pq")
                    pts[m] = pt
                for k in range(NK):
                    for m in ms:
                        for nh in range(2):
                            nc.tensor.matmul(
                                pts[m][:, nh * 512 : (nh + 1) * 512],
                                wtiles[m][:, k * PART : (k + 1) * PART],
                                xt[k][:, nh * 512 : (nh + 1) * 512],
                                start=(k == 0),
                                stop=(k == NK - 1),
                            )
                for m in ms:
                    nc.vector.tensor_scalar_add(
                        qkvT[m][:], pts[m][:], bqkv_sb[:, m : m + 1]
                    )
            if debug:
                nc.sync.dma_start(dbg["dbg_q"][:, :], qkvT[0][:].bitcast(F32))
                nc.sync.dma_start(dbg["dbg_k"][:, :], qkvT[NLQ][:].bitcast(F32))
                nc.sync.dma_start(dbg["dbg_v"][:, :], qkvT[2 * NLQ][:].bitcast(F32))

        # ================= phase 2: attention per head =================
        with (
            tc.tile_pool(name="epool", bufs=8) as e_pool,
            tc.tile_pool(name="vtpool", bufs=2) as vt_pool,
            tc.tile_pool(name="otraw", bufs=2) as otraw_pool,
            tc.tile_pool(name="rcp", bufs=1) as rcp_pool,
            tc.tile_pool(name="pst", bufs=2, space="PSUM") as pst_pool,
            tc.tile_pool(name="po", bufs=1, space="PSUM") as po_pool,
            tc.tile_pool(name="pvt", bufs=1, space="PSUM") as pvt_pool,
        ):
            otr = None
            rcpair = None
            for h in range(H):
                j = h // 2
                qt = qkvT[j]
                kt = qkvT[NLQ + j]
                vsrc = qkvT[2 * NLQ + j]
                ro = (h % 2) * HD  # partition row offset within the pair tile

                # --- V^T (+ones col) for this head ---
                pvt = pvt_pool.tile([PART, 512], F32, tag="pvt")
                for c in range(NLK):
                    nc.tensor.transpose(
                        pvt[:, c * HD : (c + 1) * HD],
                        vsrc[ro : ro + HD, c * PART : (c + 1) * PART].bitcast(F32),
                        ident[ro : ro + HD, 0:HD],
                    )
                vt = vt_pool.tile([PART, NLK * (HD + 1)], F32R, tag="vt")
                vt3 = vt[:].rearrange("p (c w) -> p c w", w=HD + 1)
                nc.vector.tensor_copy(
                    vt3[:, :, 0:HD],
                    pvt[:].rearrange("p (c w) -> p c w", w=HD),
                )
                for c in range(NLK):
                    nc.vector.tensor_copy(
                        vt[:, c * (HD + 1) + HD : (c + 1) * (HD + 1)], ones_sb[:]
                    )

                # --- scores^T + exp ---
                etiles = []
                for c in range(NLK):
                    st = pst_pool.tile([PART, L], F32, tag="pst")
                    for nh in range(2):
                        nc.tensor.matmul(
                            st[:, nh * 512 : (nh + 1) * 512],
                            kt[ro : ro + HD, c * PART : (c + 1) * PART],
                            qt[ro : ro + HD, nh * 512 : (nh + 1) * 512],
                            start=True,
                            stop=True,
                        )
                    et = e_pool.tile([PART, L], F32R, tag="e")
                    nc.scalar.activation(
                        et[:],
                        st[:],
                        mybir.ActivationFunctionType.Exp,
                        bias=mb_sb[:, c : c + 1],
                        scale=1.0 / 8.0,
                    )
                    etiles.append(et)
                if debug and h == 0:
                    nc.sync.dma_start(dbg["dbg_e"][:, :], etiles[0][:].bitcast(F32))

                if debug and h == 0:
                    nc.sync.dma_start(dbg["dbg_vt"][:, :], vt[:].bitcast(F32))
                # --- O_un^T (+denominator row) ---
                po = po_pool.tile([PART, L], F32, tag="po")
                for c in range(NLK):
                    for nh in range(2):
                        nc.tensor.matmul(
                            po[0 : HD + 1, nh * 512 : (nh + 1) * 512],
                            vt[:, c * (HD + 1) : (c + 1) * (HD + 1)],
                            etiles[c][:, nh * 512 : (nh + 1) * 512],
                            start=(c == 0),
                            stop=(c == NLK - 1),
                        )

                # --- collect raw rows + reciprocal of denominator row ---
                if h % 2 == 0:
                    otr = otraw_pool.tile([PART, L], F32, tag="otraw")
                    rcpair = rcp_pool.tile([1, 2 * L], F32R, tag="rcp")
                if debug and h == 0:
                    dpo = otraw_pool.tile([PART, L], F32, tag="dpo")
                    nc.vector.tensor_copy(dpo[0 : HD + 1, :], po[0 : HD + 1, :])
                    nc.sync.dma_start(dbg["dbg_po"][:, :], dpo[:])
                nc.vector.tensor_copy(otr[ro : ro + HD, :], po[0:HD, :])
                with nc.allow_low_precision(reason="f32r denom reciprocal"):
                    nc.vector.reciprocal(
                        rcpair[0:1, (h % 2) * L : (h % 2 + 1) * L],
                        po[HD : HD + 1, :],
                    )

                if h % 2 == 1:
                    # place the two 1/denom rows on partitions 0/1, then
                    # broadcast across partitions with a K=2 selector matmul
                    rc2 = rcp_pool.tile([2, L], F32R, tag="rc2")
                    nc.sync.dma_start(rc2[0:1, :], rcpair[0:1, 0:L])
                    nc.sync.dma_start(rc2[1:2, :], rcpair[0:1, L : 2 * L])
                    rt = po_pool.tile([PART, L], F32, tag="po")
                    for half in range(2):
                        ns = slice(half * 512, (half + 1) * 512)
                        nc.tensor.matmul(
                            rt[:, ns], sel_sb[:], rc2[0:2, ns],
                            start=True, stop=True,
                        )
                    nc.vector.tensor_mul(ot_fin[j][:], otr[:], rt[:])
                    if debug and h == 1:
                        drt = otraw_pool.tile([PART, L], F32, tag="dpo")
                        nc.vector.tensor_copy(drt[:], rt[:])
                        nc.sync.dma_start(dbg["dbg_rt"][:, :], drt[:])
                        nc.sync.dma_start(dbg["dbg_ot"][:, :], ot_fin[j][:].bitcast(F32))

        # ================= phase 3: output projection =================
        with (
            tc.tile_pool(name="woutp", bufs=1) as wout_pool,
            tc.tile_pool(name="fsb", bufs=2) as f_pool,
            tc.tile_pool(name="pf", bufs=2, space="PSUM") as pf_pool,
        ):
            wo = []
            for k in range(NK):
                t = wout_pool.tile([PART, D], F32R, tag=f"wo{k}")
                nc.sync.dma_start(
                    t[:], wout[k * PART : (k + 1) * PART, :].bitcast(F32R)
                )
                wo.append(t)
            for lq in range(NLQ):
                pf = pf_pool.tile([PART, D], F32, tag="pf")
                for k in range(NK):
                    for nh in range(2):
                        nc.tensor.matmul(
                            pf[:, nh * 512 : (nh + 1) * 512],
                            ot_fin[k][:, lq * PART : (lq + 1) * PART],
                            wo[k][:, nh * 512 : (nh + 1) * 512],
                            start=(k == 0),
                            stop=(k == NK - 1),
                        )
                fs = f_pool.tile([PART, D], F32, tag="fsb")
                nc.vector.tensor_add(fs[:], pf[:], bout_sb[:])
                nc.sync.dma_start(Y[lq * PART : (lq + 1) * PART, :], fs[:])

    nc.compile()
    return nc


_NC_CACHE = None


def _get_nc():
    global _NC_CACHE
    if _NC_CACHE is None:
        _NC_CACHE = build_nc()
    return _NC_CACHE


def make_in_maps(x, attn_mask, w_qkv, b_qkv, w_out, b_out):
    """Host-side sharding + layout prep -> per-core input maps."""
    x = np.asarray(x, dtype=np.float32)
    attn_mask = np.asarray(attn_mask)
    w_qkv = np.asarray(w_qkv, dtype=np.float32)
    b_qkv = np.ascontiguousarray(np.asarray(b_qkv, dtype=np.float32))
    w_out = np.ascontiguousarray(np.asarray(w_out, dtype=np.float32))
    b_out = np.asarray(b_out, dtype=np.float32)

    # wqkv_blk[m, p, k, c] = w_qkv[k*128 + p, m*128 + c]
    wblk = np.ascontiguousarray(
        w_qkv.reshape(NK, PART, NM, PART).transpose(2, 1, 0, 3)
    )
    maskbias = np.where(attn_mask.astype(bool), 0.0, -10000.0).astype(np.float32)

    sel_host = np.zeros((2, PART), dtype=np.float32)
    sel_host[0, 0:HD] = 1.0
    sel_host[1, HD:PART] = 1.0
    in_maps = []
    for b in range(B):
        in_maps.append(
            {
                "xT": np.ascontiguousarray(x[b].T),
                "wqkv_blk": wblk,
                "bqkv": b_qkv,
                "wout": w_out,
                "bout": np.ascontiguousarray(np.broadcast_to(b_out, (PART, D))),
                "maskb": np.ascontiguousarray(maskbias[b]),
                "sel": sel_host,
            }
        )
    return in_maps


def kernel(x, attn_mask, w_qkv, b_qkv, w_out, b_out):
    in_maps = make_in_maps(x, attn_mask, w_qkv, b_qkv, w_out, b_out)
    nc = _get_nc()
    res = run_bass_kernel_spmd(nc, in_maps, core_ids=list(range(N_CORES)))
    return np.stack([res.results[b]["Y"] for b in range(B)], axis=0)


if __name__ == "__main__":
    rng = np.random.default_rng(0)
    inputs = {
        "x": rng.standard_normal((B, L, D), dtype=np.float32),
        "attn_mask": np.ones((B, L), dtype=bool),
        "w_qkv": ((rng.random((D, D3), dtype=np.float32) - 0.5) / 16.0),
        "b_qkv": np.zeros((D3,), dtype=np.float32),
        "w_out": ((rng.random((D, D), dtype=np.float32) - 0.5) / 16.0),
        "b_out": np.zeros((D,), dtype=np.float32),
    }
    y = kernel(**inputs)
    print(y.shape, y.dtype)


# revision 13
# speedup vs baseline: 1.1627x; 1.1627x over previous
"""Multi-head self-attention Trainium2 kernel (Bass/Tile), batch-parallel
over 8 NeuronCores.

Problem (hardcoded): B=8, L=1024, D=1024, H=16, hd=64, f32.
  qkv = x @ w_qkv + b_qkv ; per-head scores = q k^T / 8 ; mask ; softmax ;
  out = (P v) heads-merged @ w_out + b_out.

Sharding: one batch element per core (data parallel); full weights on every
core. No collectives.

Per-core dataflow (all real matmuls in float32r, N=512, f32 PSUM accum):
  - host provides xT (D x L, dim-major) and w_qkv pre-blocked so every DMA is
    contiguous.
  - qkvT[3D x L] = w_qkv^T @ x^T via 24 M-tiles x 8 K-chunks (PSUM accum),
    evacuated with per-partition b_qkv add (DVE) -> SBUF f32r.
  - per head: ST = k^T q (scores TRANSPOSED: [Lk x Lq]) -> exp via ScalarE
    with scale=1/8 and per-partition mask bias (exact masking for free),
    no max-subtraction (inputs bounded; softmax is shift-invariant).
  - V^T per head via PE transposes into one PSUM bank, augmented with a ones
    column so the attention matmul also produces the softmax denominator.
  - O_un^T[65 x Lq] = V'^T E accumulated over Lk chunks. Rows 0:64 are the
    unnormalized head output (dim-major), row 64 the denominator.
  - normalization deferred: OT chunk tiles [128 x L] collect 2 heads' raw
    rows; R = 1/denom broadcast across partitions via SBUF->SBUF DMA;
    one DVE multiply normalizes and rounds to f32r.
  - final = OT^T @ w_out per Lq-tile (+ b_out broadcast add) -> token-major
    output, DMA'd straight to DRAM.
"""

import sys

import numpy as np

try:
    import concourse.bass as bass  # noqa: F401
except Exception:  # pragma: no cover - defensive path setup
    for p in ("/opt/trn_rl_repo", "/opt/pypackages"):
        if p not in sys.path:
            sys.path.insert(0, p)
    import concourse.bass as bass  # noqa: F401

from contextlib import ExitStack

import concourse.tile as tile
from concourse import bacc, mybir
from concourse.bass_utils import run_bass_kernel_spmd
from concourse.masks import make_identity

F32 = mybir.dt.float32
F32R = mybir.dt.float32r

B, L, D = 8, 1024, 1024
H, HD = 16, 64
D3 = 3 * D
N_CORES = 8
PART = 128
NK = D // PART  # 8 contraction chunks
NM = D3 // PART  # 24 qkv output tiles
NLQ = L // PART  # 8 query tiles
NLK = L // PART  # 8 key tiles
MG = 3  # qkv M-tiles per PSUM group


def build_nc(debug=False):
    nc = bacc.Bacc("TRN2", target_bir_lowering=False, debug=False)

    xT = nc.dram_tensor("xT", (D, L), F32, kind="ExternalInput").ap()
    # w_qkv blocked on host: wqkv_blk[m, p, k, c] = w_qkv[k*128 + p, m*128 + c]
    wqkv_blk = nc.dram_tensor(
        "wqkv_blk", (NM, PART, NK, PART), F32, kind="ExternalInput"
    ).ap()
    bqkv = nc.dram_tensor("bqkv", (D3,), F32, kind="ExternalInput").ap()
    wout = nc.dram_tensor("wout", (D, D), F32, kind="ExternalInput").ap()
    bout = nc.dram_tensor("bout", (PART, D), F32, kind="ExternalInput").ap()
    maskb = nc.dram_tensor("maskb", (L,), F32, kind="ExternalInput").ap()
    sel = nc.dram_tensor("sel", (2, PART), F32, kind="ExternalInput").ap()
    Y = nc.dram_tensor("Y", (L, D), F32, kind="ExternalOutput").ap()
    dbg = {}
    if debug:
        for nm, shp in [
            ("dbg_q", (PART, L)), ("dbg_k", (PART, L)), ("dbg_v", (PART, L)),
            ("dbg_e", (PART, L)), ("dbg_vt", (PART, NLK * (HD + 1))),
            ("dbg_po", (PART, L)), ("dbg_rt", (PART, L)), ("dbg_ot", (PART, L)),
        ]:
            dbg[nm] = nc.dram_tensor(nm, shp, F32, kind="ExternalOutput").ap()

    with tile.TileContext(nc) as tc, ExitStack() as ctx:
        singles = ctx.enter_context(tc.tile_pool(name="singles", bufs=1))

        # two stacked 64x64 identities so V-transposes work from either
        # base partition (matmul requires lhsT/rhs at the same base)
        ident = singles.tile([PART, HD], F32)
        make_identity(nc, ident[0:HD, 0:HD])
        make_identity(nc, ident[HD:PART, 0:HD])
        ones_sb = singles.tile([PART, 1], F32)
        nc.vector.memset(ones_sb[:], 1.0)
        sel_sb = singles.tile([2, PART], F32R)
        nc.sync.dma_start(sel_sb[:], sel[:, :].bitcast(F32R))
        bqkv_sb = singles.tile([PART, NM], F32)
        nc.sync.dma_start(bqkv_sb[:], bqkv.rearrange("(c p) -> p c", p=PART))
        mb_sb = singles.tile([PART, NLK], F32)
        nc.sync.dma_start(mb_sb[:], maskb.rearrange("(c p) -> p c", p=PART))
        bout_sb = singles.tile([PART, D], F32)

        # ---- tiles that must survive across phases ----
        qkvT_pool = ctx.enter_context(tc.tile_pool(name="qkvT", bufs=1))
        qkvT = []
        for m in range(NM):
            t = qkvT_pool.tile([PART, L], F32R, tag=f"qkvT{m}")
            qkvT.append(t)

        ot_pool = ctx.enter_context(tc.tile_pool(name="otpool", bufs=1))
        ot_fin = []
        for j in range(NK):
            t = ot_pool.tile([PART, L], F32R, tag=f"ot{j}")
            ot_fin.append(t)

        # ================= phase 1: qkv projection =================
        with (
            tc.tile_pool(name="xt", bufs=1) as xt_pool,
            tc.tile_pool(name="wblk", bufs=2 * MG) as wblk_pool,
            tc.tile_pool(name="pq", bufs=MG, space="PSUM") as pq_pool,
        ):
            xt = []
            for k in range(NK):
                t = xt_pool.tile([PART, L], F32R, tag=f"xt{k}")
                xt.append(t)

            def load_xt(k):
                nc.sync.dma_start(
                    xt[k][:], xT[k * PART : (k + 1) * PART, :].bitcast(F32R)
                )

            xt_loaded = 0
            for g in range(NM // MG):
                ms = [g * MG + i for i in range(MG)]
                wtiles = {}
                for m in ms:
                    wt = wblk_pool.tile([PART, NK * PART], F32R, tag="wblk")
                    nc.sync.dma_start(
                        wt[:],
                        wqkv_blk[m].rearrange("p k c -> p (k c)").bitcast(F32R),
                    )
                    wtiles[m] = wt
                pts = {}
                for m in ms:
                    pt = pq_pool.tile([PART, L], F32, tag="pq")
                    pts[m] = pt
                for k in range(NK):
                    while xt_loaded < min(NK, k + 2):
                        load_xt(xt_loaded)
                        xt_loaded += 1
                    for m in ms:
                        for nh in range(2):
                            nc.tensor.matmul(
                                pts[m][:, nh * 512 : (nh + 1) * 512],
                                wtiles[m][:, k * PART : (k + 1) * PART],
                                xt[k][:, nh * 512 : (nh + 1) * 512],
                                start=(k == 0),
                                stop=(k == NK - 1),
                            )
                for m in ms:
                    nc.scalar.activation(
                        qkvT[m][:],
                        pts[m][:],
                        mybir.ActivationFunctionType.Identity,
                        bias=bqkv_sb[:, m : m + 1],
                        scale=1.0,
                    )
            if debug:
                nc.sync.dma_start(dbg["dbg_q"][:, :], qkvT[0][:].bitcast(F32))
                nc.sync.dma_start(dbg["dbg_k"][:, :], qkvT[NLQ][:].bitcast(F32))
                nc.sync.dma_start(dbg["dbg_v"][:, :], qkvT[2 * NLQ][:].bitcast(F32))

        # ================= phase 2: attention per head =================
        with (
            tc.tile_pool(name="epool", bufs=8) as e_pool,
            tc.tile_pool(name="vtpool", bufs=2) as vt_pool,
            tc.tile_pool(name="otraw", bufs=2) as otraw_pool,
            tc.tile_pool(name="rcp", bufs=2) as rcp_pool,
            tc.tile_pool(name="pst", bufs=2, space="PSUM") as pst_pool,
            tc.tile_pool(name="po", bufs=2, space="PSUM") as po_pool,
        ):
            otr = None
            rcpair = None
            pending = None
            for h in range(H):
                j = h // 2
                qt = qkvT[j]
                kt = qkvT[NLQ + j]
                vsrc = qkvT[2 * NLQ + j]
                ro = (h % 2) * HD  # partition row offset within the pair tile

                # --- V^T (+ones col) for this head ---
                pvt = pst_pool.tile([PART, 512], F32, tag="pst")
                for c in range(NLK):
                    nc.tensor.transpose(
                        pvt[:, c * HD : (c + 1) * HD],
                        vsrc[ro : ro + HD, c * PART : (c + 1) * PART].bitcast(F32),
                        ident[ro : ro + HD, 0:HD],
                    )
                vt = vt_pool.tile([PART, NLK * (HD + 1)], F32R, tag="vt")
                vt3 = vt[:].rearrange("p (c w) -> p c w", w=HD + 1)
                nc.vector.tensor_copy(
                    vt3[:, :, 0:HD],
                    pvt[:].rearrange("p (c w) -> p c w", w=HD),
                )
                for c in range(NLK):
                    nc.vector.tensor_copy(
                        vt[:, c * (HD + 1) + HD : (c + 1) * (HD + 1)], ones_sb[:]
                    )

                # --- scores^T + exp ---
                etiles = []
                for c in range(NLK):
                    st = pst_pool.tile([PART, L], F32, tag="pst")
                    for nh in range(2):
                        nc.tensor.matmul(
                            st[:, nh * 512 : (nh + 1) * 512],
                            kt[ro : ro + HD, c * PART : (c + 1) * PART],
                            qt[ro : ro + HD, nh * 512 : (nh + 1) * 512],
                            start=True,
                            stop=True,
                        )
                    et = e_pool.tile([PART, L], F32R, tag="e")
                    nc.scalar.activation(
                        et[:],
                        st[:],
                        mybir.ActivationFunctionType.Exp,
                        bias=mb_sb[:, c : c + 1],
                        scale=1.0 / 8.0,
                    )
                    etiles.append(et)
                if debug and h == 0:
                    nc.sync.dma_start(dbg["dbg_e"][:, :], etiles[0][:].bitcast(F32))

                if debug and h == 0:
                    nc.sync.dma_start(dbg["dbg_vt"][:, :], vt[:].bitcast(F32))
                if pending is not None:
                    pj, potr, prc2 = pending
                    rt = po_pool.tile([PART, L], F32, tag="po")
                    for half in range(2):
                        ns = slice(half * 512, (half + 1) * 512)
                        nc.tensor.matmul(
                            rt[:, ns], sel_sb[:], prc2[0:2, ns],
                            start=True, stop=True,
                        )
                    nc.vector.tensor_mul(ot_fin[pj][:], potr[:], rt[:])
                    pending = None

                # --- O_un^T (+denominator row) ---
                po = po_pool.tile([PART, L], F32, tag="po")
                for c in range(NLK):
                    for nh in range(2):
                        nc.tensor.matmul(
                            po[0 : HD + 1, nh * 512 : (nh + 1) * 512],
                            vt[:, c * (HD + 1) : (c + 1) * (HD + 1)],
                            etiles[c][:, nh * 512 : (nh + 1) * 512],
                            start=(c == 0),
                            stop=(c == NLK - 1),
                        )

                # --- collect raw rows + reciprocal of denominator row ---
                if h % 2 == 0:
                    otr = otraw_pool.tile([PART, L], F32, tag="otraw")
                    rcpair = rcp_pool.tile([1, 2 * L], F32R, tag="rcp")
                if debug and h == 0:
                    dpo = otraw_pool.tile([PART, L], F32, tag="dpo")
                    nc.vector.tensor_copy(dpo[0 : HD + 1, :], po[0 : HD + 1, :])
                    nc.sync.dma_start(dbg["dbg_po"][:, :], dpo[:])
                nc.vector.tensor_copy(otr[ro : ro + HD, :], po[0:HD, :])
                with nc.allow_low_precision(reason="f32r denom reciprocal"):
                    nc.vector.reciprocal(
                        rcpair[0:1, (h % 2) * L : (h % 2 + 1) * L],
                        po[HD : HD + 1, :],
                    )

                if h % 2 == 1:
                    # place the two 1/denom rows on partitions 0/1; the
                    # selector-matmul broadcast + normalize runs after the
                    # NEXT head's scores so the PE never waits on this DMA
                    rc2 = rcp_pool.tile([2, L], F32R, tag="rc2")
                    nc.sync.dma_start(rc2[0:1, :], rcpair[0:1, 0:L])
                    nc.sync.dma_start(rc2[1:2, :], rcpair[0:1, L : 2 * L])
                    pending = (j, otr, rc2)
                    if debug and h == 1:
                        drt = otraw_pool.tile([PART, L], F32, tag="dpo")
                        nc.vector.tensor_copy(drt[:], rt[:])
                        nc.sync.dma_start(dbg["dbg_rt"][:, :], drt[:])
                        nc.sync.dma_start(dbg["dbg_ot"][:, :], ot_fin[j][:].bitcast(F32))

            if pending is not None:
                pj, potr, prc2 = pending
                rt = po_pool.tile([PART, L], F32, tag="po")
                for half in range(2):
                    ns = slice(half * 512, (half + 1) * 512)
                    nc.tensor.matmul(
                        rt[:, ns], sel_sb[:], prc2[0:2, ns],
                        start=True, stop=True,
                    )
                nc.vector.tensor_mul(ot_fin[pj][:], potr[:], rt[:])
                pending = None

        # ================= phase 3: output projection =================
        with (
            tc.tile_pool(name="woutp", bufs=1) as wout_pool,
            tc.tile_pool(name="fsb", bufs=2) as f_pool,
            tc.tile_pool(name="pf", bufs=2, space="PSUM") as pf_pool,
        ):
            nc.sync.dma_start(bout_sb[:], bout[:, :])
            wo = []
            for k in range(NK):
                t = wout_pool.tile([PART, D], F32R, tag=f"wo{k}")
                nc.sync.dma_start(
                    t[:], wout[k * PART : (k + 1) * PART, :].bitcast(F32R)
                )
                wo.append(t)
            for lq in range(NLQ):
                pf = pf_pool.tile([PART, D], F32, tag="pf")
                for k in range(NK):
                    for nh in range(2):
                        nc.tensor.matmul(
                            pf[:, nh * 512 : (nh + 1) * 512],
                            ot_fin[k][:, lq * PART : (lq + 1) * PART],
                            wo[k][:, nh * 512 : (nh + 1) * 512],
                            start=(k == 0),
                            stop=(k == NK - 1),
                        )
                fs = f_pool.tile([PART, D], F32, tag="fsb")
                nc.vector.tensor_add(fs[:], pf[:], bout_sb[:])
                nc.sync.dma_start(Y[lq * PART : (lq + 1) * PART, :], fs[:])

    nc.compile()
    return nc


_NC_CACHE = None


def _get_nc():
    global _NC_CACHE
    if _NC_CACHE is None:
        _NC_CACHE = build_nc()
    return _NC_CACHE


def make_in_maps(x, attn_mask, w_qkv, b_qkv, w_out, b_out):
    """Host-side sharding + layout prep -> per-core input maps."""
    x = np.asarray(x, dtype=np.float32)
    attn_mask = np.asarray(attn_mask)
    w_qkv = np.asarray(w_qkv, dtype=np.float32)
    b_qkv = np.ascontiguousarray(np.asarray(b_qkv, dtype=np.float32))
    w_out = np.ascontiguousarray(np.asarray(w_out, dtype=np.float32))
    b_out = np.asarray(b_out, dtype=np.float32)

    # wqkv_blk[m, p, k, c] = w_qkv[k*128 + p, m*128 + c]
    wblk = np.ascontiguousarray(
        w_qkv.reshape(NK, PART, NM, PART).transpose(2, 1, 0, 3)
    )
    maskbias = np.where(attn_mask.astype(bool), 0.0, -10000.0).astype(np.float32)

    sel_host = np.zeros((2, PART), dtype=np.float32)
    sel_host[0, 0:HD] = 1.0
    sel_host[1, HD:PART] = 1.0
    in_maps = []
    for b in range(B):
        in_maps.append(
            {
                "xT": np.ascontiguousarray(x[b].T),
                "wqkv_blk": wblk,
                "bqkv": b_qkv,
                "wout": w_out,
                "bout": np.ascontiguousarray(np.broadcast_to(b_out, (PART, D))),
                "maskb": np.ascontiguousarray(maskbias[b]),
                "sel": sel_host,
            }
        )
    return in_maps


def kernel(x, attn_mask, w_qkv, b_qkv, w_out, b_out):
    in_maps = make_in_maps(x, attn_mask, w_qkv, b_qkv, w_out, b_out)
    nc = _get_nc()
    res = run_bass_kernel_spmd(nc, in_maps, core_ids=list(range(N_CORES)))
    return np.stack([res.results[b]["Y"] for b in range(B)], axis=0)


if __name__ == "__main__":
    rng = np.random.default_rng(0)
    inputs = {
        "x": rng.standard_normal((B, L, D), dtype=np.float32),
        "attn_mask": np.ones((B, L), dtype=bool),
        "w_qkv": ((rng.random((D, D3), dtype=np.float32) - 0.5) / 16.0),
        "b_qkv": np.zeros((D3,), dtype=np.float32),
        "w_out": ((rng.random((D, D), dtype=np.float32) - 0.5) / 16.0),
        "b_out": np.zeros((D,), dtype=np.float32),
    }
    y = kernel(**inputs)
    print(y.shape, y.dtype)
